# revision 2
# baseline (speedup 1.0000x reference)
"""Multi-head attention v2 on 8 TRN2 NeuronCores.

Sharding: batch (2) x head-pairs (4) -> 8 cores (as v1): each core computes q,k,v
for its 2 heads of its batch, full attention over the 4096-token sequence, and a
partial row-sharded output projection; the host sums 4 partials per batch + b_proj.

v2 changes vs v1:
  - AV is p-STATIONARY: out_av[s_blk, 65] = sum_t p[t, s_blk]^T @ [v_t | 1].
    Streams v (65 rows/matmul) instead of p (512), halving PE row traffic for AV;
    the ones column lands the softmax denominator in out column 64.
  - Normalization is a per-partition scalar multiply (denominator is a psum
    column), killing v1's reciprocal->DRAM->broadcast-DMA->mul chain.
  - exp is split across engines: ScalarE does exact exp (scaled by E[rho] via the
    activation bias so branches match), DVE does a Schraudolph int16 bitcast
    approximation on a ~3/8 subset of key-chunks (one tensor_scalar op/tile).
  - attention output [s, hd] is PE-transposed back to [hd, s] for the projection.
"""
import math
from contextlib import ExitStack

import ml_dtypes
import numpy as np

import concourse.bass as bass
import concourse.tile as tile
from concourse import bacc, mybir
from concourse.bass_utils import run_bass_kernel_spmd

B, S, D = 2, 4096, 512
H, HD = 8, 64
SCALE = HD**-0.5
P = 128
SC = 512            # query-chunk width
N_SC = S // SC      # 8
N_T = S // P        # 32 key chunks
KT = D // P         # 4 contraction tiles for the projections
VW = 132            # v row: [vA(64) | 1 | pad | vB(64) | 1 | pad]
VB0 = 66            # head-B offset inside v rows
BF16 = mybir.dt.bfloat16
F32 = mybir.dt.float32
I16 = mybir.dt.int16
EXP = mybir.ActivationFunctionType.Exp

# Schraudolph exp on DVE: bf16 bits = trunc(x*SCALE*128/ln2 + (127*128 + 0.5)).
# Mean ratio schraud/exp over uniform mantissa fraction is E=1.0406934; the
# exact-exp branch is inflated by the same factor via the activation bias so
# both branches agree in expectation (softmax cancels the common factor).
A_DVE = SCALE * 128.0 / math.log(2.0)
B_DVE = 127.0 * 128.0 + 0.5
EXP_BIAS = math.log(1.0406934)

DVE_SET = (2, 5, 7)   # t%8 routed to DVE-schraudolph (phi = 3/8)
# per-(t%8) exp route: A=ScalarE exact exp, D=DVE schraudolph,
# P=GPSIMD schraudolph with DVE psum->sbuf staging, Q=same with ScalarE staging
ROUTE = None          # derived from dve_set when None

_NC = None


def _emit(tc, out_d, xT_d, wq_d, wk_d, wv_d, bq_d, bk_d, bv_d, wp_d, id_d,
          reps=1, hw_loop=0, dve_set=DVE_SET, route=None,
          exp_half=False, av_half=False, tevac_act=0, pevac_act=0,
          av_first=True, proj_slack=True, no_exp=False, no_av=False,
          sc_half=False, stream_av=False, n_pop=None, pe_warm=0, sc_bufs=2):
    if route is None:
        route = "".join("D" if i in dve_set else "A" for i in range(8))
    nc = tc.nc
    with ExitStack() as ctx:
        consts = ctx.enter_context(tc.tile_pool(name="consts", bufs=1))
        big = ctx.enter_context(tc.tile_pool(name="big", bufs=1))
        xt_pool = ctx.enter_context(tc.tile_pool(name="xt", bufs=2))
        work = ctx.enter_context(tc.tile_pool(name="work", bufs=2))

        wq_sb = consts.tile([P, KT, P], BF16, tag="wq")
        nc.sync.dma_start(wq_sb[:], wq_d.rearrange("(kt p) m -> p kt m", p=P))
        wk_sb = consts.tile([P, KT, P], BF16, tag="wk")
        nc.sync.dma_start(wk_sb[:], wk_d.rearrange("(kt p) m -> p kt m", p=P))
        wv_sb = consts.tile([P, KT, VW], BF16, tag="wv")
        nc.sync.dma_start(wv_sb[:], wv_d.rearrange("(kt p) m -> p kt m", p=P))
        wp_sb = consts.tile([P, D], BF16, tag="wp")
        nc.sync.dma_start(wp_sb[:], wp_d)
        bq_sb = consts.tile([P, 1], F32, tag="bq")
        nc.sync.dma_start(bq_sb[:], bq_d)
        bk_sb = consts.tile([P, 1], F32, tag="bk")
        nc.sync.dma_start(bk_sb[:], bk_d)
        bv_sb = consts.tile([P, VW], F32, tag="bv")
        nc.sync.dma_start(bv_sb[:], bv_d)
        id_sb = consts.tile([P, P], BF16, tag="ident")
        nc.sync.dma_start(id_sb[:], id_d)
        ones_sb = consts.tile([P, HD], F32, tag="ones")
        nc.vector.memset(ones_sb[:], 1.0)
        eb_sb = consts.tile([P, 1], F32, tag="expbias")
        nc.vector.memset(eb_sb[:], EXP_BIAS)

        qT_sb = big.tile([P, S], BF16, tag="qT")   # rows 0-63 head A, 64-127 head B
        kT_sb = big.tile([P, S], BF16, tag="kT")
        v_sb = big.tile([P, N_T, VW], BF16, tag="v")
        # exp'd scoresT per head, double-buffered by query-chunk parity so the
        # AV pass over chunk sc-1 never races the exp writes of chunk sc
        pAB2 = [big.tile([P, N_T, 2, SC], BF16, tag="p0", name="p0"),
                big.tile([P, N_T, 2, SC], BF16, tag="p1", name="p1")]
        if no_exp:
            nc.vector.memset(pAB2[0][:], 1.0)
            nc.vector.memset(pAB2[1][:], 1.0)

        xT_r = xT_d.rearrange("(kt p) s -> p kt s", p=P)

        # psum budget is 8 banks: score tiles are 2 banks each. At sc_bufs=3
        # (chain depth 3: exp(t) gates scores(t+3) instead of t+2) the av and
        # misc pools drop to single-buffer to fit 3*2 + 1 + 1 = 8.
        sc_ps = ctx.enter_context(tc.tile_pool(name="scps", bufs=4 if sc_half else sc_bufs,
                                               space="PSUM"))
        av_ps = ctx.enter_context(tc.tile_pool(name="avps", bufs=1 if sc_bufs > 2 else 2,
                                               space="PSUM"))
        mi_ps = ctx.enter_context(tc.tile_pool(name="mips", bufs=1 if sc_bufs > 2 else 2,
                                               space="PSUM"))

        def load_x(c, tag="xt"):
            csl = slice(c * SC, (c + 1) * SC)
            xt = xt_pool.tile([P, KT, SC], BF16, tag=tag, name=tag)
            nc.sync.dma_start(xt[:], xT_r[:, :, csl])
            return xt

        def k_chunk(c, xt):
            csl = slice(c * SC, (c + 1) * SC)
            kp = mi_ps.tile([P, SC], F32, tag="m", name="kp")
            for kt in range(KT):
                nc.tensor.matmul(kp[:], lhsT=wk_sb[:, kt, :], rhs=xt[:, kt, :],
                                 start=kt == 0, stop=kt == KT - 1)
            nc.vector.tensor_scalar_add(out=kT_sb[:, csl], in0=kp[:], scalar1=bk_sb[:])

        def q_chunk(c, xt):
            csl = slice(c * SC, (c + 1) * SC)
            qp = mi_ps.tile([P, SC], F32, tag="m", name="qp")
            for kt in range(KT):
                nc.tensor.matmul(qp[:], lhsT=wq_sb[:, kt, :], rhs=xt[:, kt, :],
                                 start=kt == 0, stop=kt == KT - 1)
            nc.vector.tensor_scalar_add(out=qT_sb[:, csl], in0=qp[:], scalar1=bq_sb[:])

        def v_chunk(c, xt, pair):
            for st in (2 * pair, 2 * pair + 1):
                vp = mi_ps.tile([P, VW], F32, tag="m", name="vp")
                for kt in range(KT):
                    nc.tensor.matmul(vp[:], lhsT=xt[:, kt, st * P:(st + 1) * P],
                                     rhs=wv_sb[:, kt, :],
                                     start=kt == 0, stop=kt == KT - 1)
                nc.vector.tensor_add(out=v_sb[:, c * 4 + st, :], in0=vp[:], in1=bv_sb[:])

        def scores_half(sc, t, h):
            # one head's scores in a 1-bank psum tile: pipeline depth 4 so the
            # exp(t) -> scores(t+2) -> exp(t+2) WAR chain stops pacing the sweep
            ssl = slice(sc * SC, (sc + 1) * SC)
            tsl = slice(t * P, (t + 1) * P)
            pAB = pAB2[sc % 2]
            sH = sc_ps.tile([P, SC], F32, tag="s", name="sH")
            nc.tensor.matmul(sH[:], lhsT=kT_sb[h * HD:(h + 1) * HD, tsl],
                             rhs=qT_sb[h * HD:(h + 1) * HD, ssl],
                             start=True, stop=True)
            if no_exp:
                return
            ew = SC // 2 if exp_half else SC
            r = route[t % 8]
            if r == "D":
                nc.vector.tensor_scalar(
                    out=pAB[:, t, h, 0:ew].bitcast(I16), in0=sH[:, 0:ew],
                    scalar1=A_DVE, scalar2=B_DVE,
                    op0=mybir.AluOpType.mult, op1=mybir.AluOpType.add)
            else:
                nc.scalar.activation(out=pAB[:, t, h, 0:ew], in_=sH[:, 0:ew],
                                     func=EXP, scale=SCALE, bias=eb_sb[:])

        def scores_t(sc, t):
            if sc_half:
                scores_half(sc, t, 0)
                scores_half(sc, t, 1)
                return
            ssl = slice(sc * SC, (sc + 1) * SC)
            tsl = slice(t * P, (t + 1) * P)
            pAB = pAB2[sc % 2]
            sAB = sc_ps.tile([P, 2, SC], F32, tag="s", name="sAB")
            nc.tensor.matmul(sAB[:, 0, :], lhsT=kT_sb[0:HD, tsl],
                             rhs=qT_sb[0:HD, ssl], start=True, stop=True)
            nc.tensor.matmul(sAB[:, 1, :], lhsT=kT_sb[HD:P, tsl],
                             rhs=qT_sb[HD:P, ssl], start=True, stop=True)
            if no_exp:
                return
            ew = SC // 2 if exp_half else SC
            if route == "mod3":
                # align the DVE tiles with one t%3 residue class: at chain
                # depth 3 each exp(t)->scores(t+3)->exp(t+3) chain then runs on
                # a single engine, so DVE queue latency never stalls ACT chains
                r = "D" if t % 3 == 2 else "A"
            else:
                r = route[t % 8]
            if r == "D":
                nc.vector.tensor_scalar(
                    out=pAB[:, t, :, 0:ew].bitcast(I16), in0=sAB[:, :, 0:ew],
                    scalar1=A_DVE, scalar2=B_DVE,
                    op0=mybir.AluOpType.mult, op1=mybir.AluOpType.add)
            elif r in ("P", "Q"):
                stg = work.tile([P, 2, SC], F32, tag="stg", name="stg")
                if r == "P":
                    nc.vector.tensor_copy(stg[:, :, 0:ew], sAB[:, :, 0:ew])
                else:
                    nc.scalar.copy(stg[:, :, 0:ew], sAB[:, :, 0:ew])
                nc.gpsimd.tensor_scalar(
                    out=pAB[:, t, :, 0:ew].bitcast(I16), in0=stg[:, :, 0:ew],
                    scalar1=A_DVE, scalar2=B_DVE,
                    op0=mybir.AluOpType.mult, op1=mybir.AluOpType.add)
            else:
                nc.scalar.activation(out=pAB[:, t, :, 0:ew], in_=sAB[:, :, 0:ew],
                                     func=EXP, scale=SCALE, bias=eb_sb[:])

        # ---- streaming AV (v-ext stationary, p moving): LDWEIGHTS fully hidden
        # under the 512-wide p stream; denominators land in psum row 64 and are
        # folded in AFTER the per-head projection as per-partition scalars.
        def av_actions_stream(scp):
            acts = []
            pAB = pAB2[scp % 2]
            aw = HD + 1 if not av_half else (HD + 1) // 2
            avx = [None, None]      # psum [HD+1, SC] per head
            avs = [None, None]      # sbuf bf16 [HD, SC] per head
            rr = [None, None]       # sbuf f32 [1, SC] reciprocal rows
            z = [None]              # sbuf f32 [P, 8] z columns (h*4+sb)
            tmp = [None]

            for h in range(2):
                def alloc(h=h):
                    avx[h] = av_ps.tile([HD + 1, SC], F32, tag="av", name="av")
                acts.append(alloc)
                for tp in range(N_T):
                    def mm(h=h, tp=tp):
                        v0 = VB0 if h else 0
                        nc.tensor.matmul(avx[h][0:aw, :],
                                         lhsT=v_sb[:, tp, v0:v0 + aw],
                                         rhs=pAB[:, tp, h, :],
                                         start=tp == 0, stop=tp == N_T - 1)
                    acts.append(mm)

            def evac0():
                avs[0] = work.tile([P, SC], BF16, tag="avs", name="avs")
                nc.vector.tensor_copy(avs[0][0:HD, :], avx[0][0:HD, :])
                rr[0] = work.tile([1, SC], F32, tag="rr", name="rr")
                nc.vector.reciprocal(rr[0][:], avx[0][HD:HD + 1, :])
            acts.append(evac0)

            def evac1():
                nc.vector.tensor_copy(avs[0][HD:P, :], avx[1][0:HD, :])
                rr[1] = work.tile([1, SC], F32, tag="rr", name="rr")
                nc.vector.reciprocal(rr[1][:], avx[1][HD:HD + 1, :])
            acts.append(evac1)

            def ztrans():
                zps = mi_ps.tile([P, 8], F32, tag="m", name="zps")
                for h in range(2):
                    bp = rr[h].base_partition()
                    for sb in range(4):
                        nc.tensor.matmul(
                            zps[:, h * 4 + sb:h * 4 + sb + 1],
                            lhsT=rr[h][:, sb * P:(sb + 1) * P],
                            rhs=ones_sb[bp:bp + 1, 0:1], start=True, stop=True)
                z[0] = (zps,)
            acts.append(ztrans)

            def zevac():
                z_sb = work.tile([P, 8], F32, tag="z", name="z")
                nc.vector.tensor_copy(z_sb[:], z[0][0][:])
                z[0] = (z_sb,)
            acts.append(zevac)

            for sb in range(4):
                def projA(sb=sb):
                    op = mi_ps.tile([P, D], F32, tag="m", name="op")
                    nc.tensor.matmul(op[:],
                                     lhsT=avs[0][0:HD, sb * P:(sb + 1) * P],
                                     rhs=wp_sb[0:HD, :], start=True, stop=True)
                    tmp[0] = (op,)
                acts.append(projA)

                def combA(sb=sb):
                    t_sb = work.tile([P, D], F32, tag="tmp", name="tmp")
                    nc.vector.tensor_scalar(
                        out=t_sb[:], in0=tmp[0][0][:],
                        scalar1=z[0][0][:, sb:sb + 1], scalar2=None,
                        op0=mybir.AluOpType.mult)
                    tmp[0] = (t_sb,)
                acts.append(combA)

                def projB(sb=sb):
                    op = mi_ps.tile([P, D], F32, tag="m", name="op")
                    nc.tensor.matmul(op[:],
                                     lhsT=avs[0][HD:P, sb * P:(sb + 1) * P],
                                     rhs=wp_sb[HD:P, :], start=True, stop=True)
                    tmp[0] = (tmp[0][0], op)
                acts.append(projB)

                def combB(scp=scp, sb=sb):
                    ot = work.tile([P, D], F32, tag="ot", name="ot")
                    nc.vector.scalar_tensor_tensor(
                        out=ot[:], in0=tmp[0][1][:],
                        scalar=z[0][0][:, 4 + sb:4 + sb + 1], in1=tmp[0][0][:],
                        op0=mybir.AluOpType.mult, op1=mybir.AluOpType.add)
                    r0 = scp * SC + sb * P
                    nc.sync.dma_start(out_d[r0:r0 + P, :], ot[:])
                acts.append(combB)
            return acts

        # ---- AV / norm / transpose / proj for one query chunk, as an action list
        def av_actions(scp):
            if no_av:
                return []
            if stream_av:
                return av_actions_stream(scp)
            acts = []
            deferred_proj = []   # proj(sb) issues ~1/4 into sb+1's block
            pAB = pAB2[scp % 2]
            aw = HD + 1 if not av_half else (HD + 1) // 2
            for sb in range(4):
                if deferred_proj and proj_slack:
                    insert_at = len(acts) + 12
                else:
                    insert_at = None
                av = [None]

                def alloc_mm(sb=sb, av=av):
                    av[0] = av_ps.tile([P, 2, HD + 1], F32, tag="av", name="av")

                acts.append(alloc_mm)
                # heads sequential: two accumulation groups can't share a psum
                # bank concurrently (zero-region granularity is a full bank)
                for h in range(2):
                    for tp in range(N_T):
                        def mm(sb=sb, h=h, tp=tp, av=av):
                            psl = slice(sb * P, (sb + 1) * P)
                            v0 = VB0 if h else 0
                            nc.tensor.matmul(
                                av[0][:, h, 0:aw],
                                lhsT=pAB[:, tp, h, psl],
                                rhs=v_sb[:, tp, v0:v0 + aw],
                                start=tp == 0, stop=tp == N_T - 1)
                        acts.append(mm)

                avn = [None]

                def norm(sb=sb, av=av, avn=avn):
                    avn[0] = work.tile([P, P], BF16, tag="avn", name="avn")
                    rec = work.tile([P, 2], F32, tag="rec", name="rec")
                    nc.vector.reciprocal(rec[:], av[0][:, :, HD])
                    nc.vector.tensor_scalar(
                        out=avn[0][:, 0:HD], in0=av[0][:, 0, 0:HD],
                        scalar1=rec[:, 0:1], scalar2=None,
                        op0=mybir.AluOpType.mult)
                    nc.vector.tensor_scalar(
                        out=avn[0][:, HD:P], in0=av[0][:, 1, 0:HD],
                        scalar1=rec[:, 1:2], scalar2=None,
                        op0=mybir.AluOpType.mult)
                acts.append(norm)

                avnT = [None]

                def transp(avn=avn, avnT=avnT):
                    tp_ps = mi_ps.tile([P, P], BF16, tag="m", name="tp")
                    nc.tensor.transpose(tp_ps[:], avn[0][:], id_sb[:])
                    avnT[0] = (tp_ps,)
                acts.append(transp)

                def tevac(sb=sb, avnT=avnT):
                    t_sb = work.tile([P, P], BF16, tag="avnT", name="avnT")
                    if (tevac_act == 2) or (tevac_act == 1 and sb % 2):
                        nc.scalar.copy(t_sb[:], avnT[0][0][:])
                    else:
                        nc.vector.tensor_copy(t_sb[:], avnT[0][0][:])
                    avnT[0] = (t_sb,)
                acts.append(tevac)

                def proj(scp=scp, sb=sb, avnT=avnT):
                    op = mi_ps.tile([P, D], F32, tag="m", name="op")
                    nc.tensor.matmul(op[:], lhsT=avnT[0][0][:], rhs=wp_sb[:],
                                     start=True, stop=True)
                    ot = work.tile([P, D], F32, tag="ot", name="ot")
                    if (pevac_act == 2) or (pevac_act == 1 and sb % 2):
                        nc.scalar.copy(ot[:], op[:])
                    else:
                        nc.vector.tensor_copy(ot[:], op[:])
                    r0 = scp * SC + sb * P
                    nc.sync.dma_start(out_d[r0:r0 + P, :], ot[:])

                if insert_at is not None:
                    acts.insert(insert_at, deferred_proj.pop())
                if proj_slack:
                    deferred_proj.append(proj)
                else:
                    acts.append(proj)
            if deferred_proj:
                acts.append(deferred_proj.pop())
            return acts

        # PE warm-up burst while the first x DMA is in flight
        for _w in range(10):
            warm = mi_ps.tile([HD, HD], F32, tag="m", name="warm")
            nc.tensor.matmul(warm[:], lhsT=ones_sb[:, 0:HD], rhs=ones_sb[:, 0:HD],
                             start=True, stop=True)

        def body():
            xt0 = load_x(0)
            xt_nxt = [load_x(1)]
            k_chunk(0, xt0)
            q_chunk(0, xt0)
            v_chunk(0, xt0, 0)
            v_chunk(0, xt0, 1)

            pending = []        # av actions of the previous chunk
            xt_cur = [None]
            xq = [None]
            for sc in range(N_SC):
                for t in range(N_T):
                    if sc == 0:
                        c = t // 4 + 1
                        if c <= N_SC - 1:
                            if t % 4 == 0:
                                xt_cur[0] = xt_nxt[0]
                                k_chunk(c, xt_cur[0])
                            elif t % 4 == 1:
                                v_chunk(c, xt_cur[0], 0)
                            elif t % 4 == 2:
                                v_chunk(c, xt_cur[0], 1)
                                if c + 1 <= N_SC - 1:
                                    xt_nxt[0] = load_x(c + 1)
                    if sc < N_SC - 1 and t == 10:
                        xq[0] = load_x(sc + 1, tag="xq")
                    if sc < N_SC - 1 and t == 20:
                        q_chunk(sc + 1, xq[0])
                    # av actions first: chain-critical DVE ops (norm/evac) enter
                    # the FIFO queue ahead of this step's exp/schraudolph
                    # dependency-free PE activity so the HAM clock-gate never
                    # throttles during exp-paced stretches where PE idles
                    for _w in range(pe_warm):
                        nc.tensor.ldweights(weights=id_sb[:, 0:HD])
                    np_ = n_pop if n_pop else (4 if stream_av else 9)
                    if av_first:
                        for _i in range(np_):
                            if pending:
                                pending.pop(0)()
                        scores_t(sc, t)
                    else:
                        scores_t(sc, t)
                        for _i in range(np_):
                            if pending:
                                pending.pop(0)()
                assert not pending, f"av actions undrained: {len(pending)}"
                pending = av_actions(sc)
            for a in pending:
                a()

        if hw_loop:
            with tc.For_i(0, hw_loop, 1):
                body()
        else:
            for _rep in range(reps):
                body()


def build_nc(reps=1, hw_loop=0, dve_set=DVE_SET, route="mod3", exp_half=False,
             av_half=False, tevac_act=2, pevac_act=0, av_first=False,
             proj_slack=False, no_exp=False, no_av=False, sc_half=False,
             stream_av=False, n_pop=None, pe_warm=0, sc_bufs=3):
    nc = bacc.Bacc("TRN2", target_bir_lowering=False, debug=False, num_devices=8)
    xT = nc.dram_tensor("xT", [D, S], BF16, kind="ExternalInput").ap()
    wq = nc.dram_tensor("wq", [D, P], BF16, kind="ExternalInput").ap()
    wk = nc.dram_tensor("wk", [D, P], BF16, kind="ExternalInput").ap()
    wv = nc.dram_tensor("wv", [D, VW], BF16, kind="ExternalInput").ap()
    bq = nc.dram_tensor("bq", [P, 1], F32, kind="ExternalInput").ap()
    bk = nc.dram_tensor("bk", [P, 1], F32, kind="ExternalInput").ap()
    bv = nc.dram_tensor("bv", [P, VW], F32, kind="ExternalInput").ap()
    wp = nc.dram_tensor("wp", [P, D], BF16, kind="ExternalInput").ap()
    ident = nc.dram_tensor("ident", [P, P], BF16, kind="ExternalInput").ap()
    out = nc.dram_tensor("out", [S, D], F32, kind="ExternalOutput").ap()
    with tile.TileContext(nc) as tc:
        _emit(tc, out, xT, wq, wk, wv, bq, bk, bv, wp, ident,
              reps=reps, hw_loop=hw_loop, dve_set=dve_set, route=route,
              exp_half=exp_half, av_half=av_half,
              tevac_act=tevac_act, pevac_act=pevac_act,
              av_first=av_first, proj_slack=proj_slack,
              no_exp=no_exp, no_av=no_av, sc_half=sc_half, stream_av=stream_av,
              n_pop=n_pop, pe_warm=pe_warm, sc_bufs=sc_bufs)
    nc.compile()
    return nc


def shard_inputs(x, W_qkv, b_qkv, W_proj):
    bf = ml_dtypes.bfloat16
    xTs = [np.ascontiguousarray(x[b].T).astype(bf) for b in range(B)]
    ident = np.eye(P, dtype=np.float32).astype(bf)
    in_maps = []
    for c in range(8):
        b, hp = divmod(c, 4)
        h0 = 2 * hp
        qc = slice(h0 * HD, h0 * HD + P)
        kc = slice(D + h0 * HD, D + h0 * HD + P)
        v0 = 2 * D + h0 * HD
        wv = np.zeros((D, VW), np.float32)
        wv[:, 0:HD] = W_qkv[:, v0:v0 + HD]
        wv[:, VB0:VB0 + HD] = W_qkv[:, v0 + HD:v0 + 2 * HD]
        bv = np.zeros((VW,), np.float32)
        bv[0:HD] = b_qkv[v0:v0 + HD]
        bv[HD] = 1.0
        bv[VB0:VB0 + HD] = b_qkv[v0 + HD:v0 + 2 * HD]
        bv[VB0 + HD] = 1.0
        in_maps.append({
            "xT": xTs[b],
            "wq": np.ascontiguousarray(W_qkv[:, qc]).astype(bf),
            "wk": np.ascontiguousarray(W_qkv[:, kc]).astype(bf),
            "wv": wv.astype(bf),
            "bq": np.ascontiguousarray(b_qkv[qc]).reshape(P, 1).astype(np.float32),
            "bk": np.ascontiguousarray(b_qkv[kc]).reshape(P, 1).astype(np.float32),
            "bv": np.tile(bv[None, :], (P, 1)).astype(np.float32),
            "wp": np.ascontiguousarray(W_proj[hp * P:(hp + 1) * P, :]).astype(bf),
            "ident": ident,
        })
    return in_maps


def kernel(x, W_qkv, b_qkv, W_proj, b_proj):
    x = np.asarray(x, np.float32)
    W_qkv = np.asarray(W_qkv, np.float32)
    b_qkv = np.asarray(b_qkv, np.float32)
    W_proj = np.asarray(W_proj, np.float32)
    b_proj = np.asarray(b_proj, np.float32)

    global _NC
    if _NC is None:
        _NC = build_nc()
    in_maps = shard_inputs(x, W_qkv, b_qkv, W_proj)
    res = run_bass_kernel_spmd(_NC, in_maps, core_ids=list(range(8)))
    outs = [r["out"].astype(np.float32) for r in res.results]
    full = np.stack([outs[4 * b] + outs[4 * b + 1] + outs[4 * b + 2] + outs[4 * b + 3] + b_proj
                     for b in range(B)])
    return full.astype(np.float32)

